# revision 1
# baseline (speedup 1.0000x reference)
"""HDC encoder kernel for Trainium2 (8 NeuronCores, data-parallel over tokens).

Math (per batch row, T tokens, D=1024):
    vecs = token_vectors[tokens]                      # [T, D], entries +-1
    s1[t] = vecs[t]
    s_o[t] = roll(s_{o-1}[t], 1) * vecs[t-(o-1)]      # o = 2..4 (zero-padded in time)
    bundle = (s1+s2+s3+s4)/4
    out = bundle @ W.T + b

Device layout (per core, 1024 tokens + 4-token front halo):
  - 9x indirect DMA gather of 128 rows each -> Vf [128 tok-part, 3, 1024] f32 x3 chunks
  - PE transposes (stride-8 column pick) -> VbT_a[p, c, col] = vec[d=8p+c] of
    gather position col (col = t' + 4), bf16.  VbT_b = VbT_a shifted one col.
  - DVE: s2/s3/s4 + 3 adds in bf16 (feature roll = c-axis shift, partition
    carry at c==0; time shift = free-axis offset).  bundle[p, c, t'].
  - PE matmul out^T[e, t'] = sum_c Wtc[:, c, e_blk].T @ bundle[:, c, t_blk],
    Wtc[p, c, e] = W[e, 8p+c]/4 (host-prepped bf16).
  - ACT copy psum->SBUF with per-partition bias (b[e]), DMA out^T to DRAM.
Host: transpose each core's out^T and assemble [B, T, E].
"""

import numpy as np
import ml_dtypes

import concourse.bass as bass
import concourse.mybir as mybir
import concourse.tile as tile
from concourse import bacc
from concourse.bass import ts, ds
from concourse.masks import make_identity
import concourse.bass_utils as bass_utils

VOCAB = 32000
D = 1024
B, T = 4, 2048
NCORES = 8
HALO = 4            # front halo slots (token a-4 .. a-1); a-4 unused, keeps 4B align
NPOS = 1024 + HALO  # valid gather positions per core
NGRP = 9            # ceil(NPOS/128)
OOB = VOCAB + 1     # index skipped by bounds_check

_cached = {}


def _build_nc():
    f32 = mybir.dt.float32
    bf16 = mybir.dt.bfloat16
    nc = bacc.Bacc("TRN2", target_bir_lowering=False, debug=False,
                   enable_asserts=False, num_devices=NCORES)

    table = nc.dram_tensor("table", [VOCAB, D], f32, kind="ExternalInput").ap()
    idx = nc.dram_tensor("idx", [128, NGRP], mybir.dt.int32, kind="ExternalInput").ap()
    wtc = nc.dram_tensor("wtc", [128, 8 * D], bf16, kind="ExternalInput").ap()
    biasb = nc.dram_tensor("biasb", [128, 8], f32, kind="ExternalInput").ap()
    prot = nc.dram_tensor("prot", [128, 128], bf16, kind="ExternalInput").ap()
    outT = nc.dram_tensor("outT", [D, 1024], f32, kind="ExternalOutput").ap()

    with tile.TileContext(nc) as tc:
        with tc.tile_pool(name="cst", bufs=1) as cst, \
             tc.tile_pool(name="vf", bufs=1) as vfp, \
             tc.tile_pool(name="vbt", bufs=1) as vbtp, \
             tc.tile_pool(name="sng", bufs=1) as sng, \
             tc.tile_pool(name="otile", bufs=3) as otp, \
             tc.tile_pool(name="pst", bufs=2, space="PSUM") as pst, \
             tc.tile_pool(name="prm", bufs=2, space="PSUM") as prm, \
             tc.tile_pool(name="psm", bufs=4, space="PSUM") as psm:

            idx_t = cst.tile([128, NGRP], mybir.dt.int32)
            nc.sync.dma_start(out=idx_t[:], in_=idx[:])
            biasb_t = cst.tile([128, 8], f32)
            nc.sync.dma_start(out=biasb_t[:], in_=biasb[:])
            wtc_t = cst.tile([128, 8, D], bf16)
            nc.sync.dma_start(out=wtc_t[:], in_=wtc.rearrange("p (c e) -> p c e", c=8))
            ident = cst.tile([128, 128], f32)
            make_identity(nc, ident[:])
            prot_t = cst.tile([128, 128], bf16)
            nc.sync.dma_start(out=prot_t[:], in_=prot[:])

            # gathered rows, token-on-partition: Vf[chunk][p, lg, d]
            vf = [vfp.tile([128, 3, D], f32, tag=f"vf{i}", name=f"vf{i}") for i in range(3)]
            # zero halo rows (batch-start cores skip them via OOB index) and
            # the never-gathered tail of the last group
            nc.gpsimd.memset(vf[0][0:HALO, 0, :], 0.0)
            nc.gpsimd.memset(vf[2][:, 2, :], 0.0)
            for g in range(NGRP):
                nc.gpsimd.indirect_dma_start(
                    out=vf[g // 3][:, g % 3, :],
                    out_offset=None,
                    in_=table,
                    in_offset=bass.IndirectOffsetOnAxis(ap=idx_t[:, g:g + 1], axis=0),
                    bounds_check=VOCAB - 1,
                    oob_is_err=False,
                )

            NCOL = NGRP * 128  # 1152 columns (gather positions)
            vbt_a = vbtp.tile([128, 8, NCOL], bf16, tag="vbta")
            vbt_b = vbtp.tile([128, 8, NCOL], bf16, tag="vbtb")

            for g in range(NGRP):
                for half in range(2):
                    ps = pst.tile([128, 512], f32, tag="tps")
                    for c4 in range(4):
                        c = 4 * half + c4
                        nc.tensor.transpose(
                            out=ps[:, ts(c4, 128)],
                            in_=vf[g // 3][:, g % 3, ds(c, 128, 8)],
                            identity=ident[:],
                        )
                    nc.scalar.copy(
                        out=vbt_a[:, 4 * half:4 * half + 4, ts(g, 128)],
                        in_=ps[:].rearrange("p (c t) -> p c t", c=4),
                    )
                # shifted copy for 4-byte alignment of odd time offsets.
                # dst window starts one col below the group so every source
                # column stays inside group g (no cross-group read -> no race
                # with the next group's transpose-copy).
                lo = 128 * g
                dst_lo = max(0, lo - 1)
                src_lo = dst_lo + 1
                w = lo + 127 - dst_lo
                nc.gpsimd.tensor_copy(
                    out=vbt_b[:, :, dst_lo:dst_lo + w],
                    in_=vbt_a[:, :, src_lo:src_lo + w],
                )

            s2 = sng.tile([128, 8, 1024], bf16, tag="s2")
            s3 = sng.tile([128, 8, 1024], bf16, tag="s3")
            s4 = sng.tile([128, 8, 1024], bf16, tag="s4")
            bnd = sng.tile([128, 8, 1024], bf16, tag="bnd")

            # Feature roll by 1 in the (p, c) d-layout: c>=1 planes shift on the
            # free c-axis; the c==0 plane needs a cyclic partition rotate,
            # done as a PE matmul with a permutation matrix (prot).
            CH = 256  # t' chunk for DVE pipelining
            mul = nc.vector.tensor_mul
            add = nc.vector.tensor_add
            for h in range(2):            # 512-wide halves
                hb = 512 * h
                H = ds(hb, 512)

                def rot(src_ap, name):
                    ps = prm.tile([128, 512], f32, tag="rot", name=name)
                    nc.tensor.matmul(out=ps[:], lhsT=prot_t[:], rhs=src_ap,
                                     start=True, stop=True)
                    sb = otp.tile([128, 512], bf16, tag="rotsb", name=name + "s")
                    nc.scalar.copy(out=sb[:], in_=ps[:])
                    return sb

                rv = rot(vbt_a[:, 7, ds(hb + 4, 512)], f"rv{h}")
                for b0 in (hb, hb + 256):
                    C = ds(b0, CH)
                    # s2 = roll(V[t'],1) * V[t'-1]   (V[t']=A@+4, V[t'-1]=B@+2)
                    mul(out=s2[:, 1:8, C], in0=vbt_a[:, 0:7, ds(b0 + 4, CH)],
                        in1=vbt_b[:, 1:8, ds(b0 + 2, CH)])
                mul(out=s2[:, 0, H], in0=rv[:], in1=vbt_b[:, 0, ds(hb + 2, 512)])

                rs2 = rot(s2[:, 7, H], f"rs2{h}")
                for b0 in (hb, hb + 256):
                    C = ds(b0, CH)
                    # s3 = roll(s2,1) * V[t'-2]      (V[t'-2]=A@+2)
                    mul(out=s3[:, 1:8, C], in0=s2[:, 0:7, C],
                        in1=vbt_a[:, 1:8, ds(b0 + 2, CH)])
                mul(out=s3[:, 0, H], in0=rs2[:], in1=vbt_a[:, 0, ds(hb + 2, 512)])

                rs3 = rot(s3[:, 7, H], f"rs3{h}")
                for b0 in (hb, hb + 256):
                    C = ds(b0, CH)
                    # s4 = roll(s3,1) * V[t'-3]      (V[t'-3]=B@+0)
                    mul(out=s4[:, 1:8, C], in0=s3[:, 0:7, C],
                        in1=vbt_b[:, 1:8, ds(b0, CH)])
                mul(out=s4[:, 0, H], in0=rs3[:], in1=vbt_b[:, 0, ds(hb, 512)])

                # bundle = V[t'] + s2 + s3 + s4
                add(out=bnd[:, :, H], in0=vbt_a[:, 0:8, ds(hb + 4, 512)],
                    in1=s2[:, :, H])
                add(out=bnd[:, :, H], in0=bnd[:, :, H], in1=s3[:, :, H])
                add(out=bnd[:, :, H], in0=bnd[:, :, H], in1=s4[:, :, H])

            # out^T[e, t'] = sum_c wtc[:, c, e_blk].T @ bundle[:, c, t_blk]
            for tb in range(2):
                for e in range(8):
                    pm = psm.tile([128, 512], f32, tag="mps")
                    for c in range(8):
                        nc.tensor.matmul(
                            out=pm[:],
                            lhsT=wtc_t[:, c, ts(e, 128)],
                            rhs=bnd[:, c, ts(tb, 512)],
                            start=(c == 0),
                            stop=(c == 7),
                        )
                    ot = otp.tile([128, 512], f32, tag="ot")
                    nc.scalar.add(out=ot[:], in_=pm[:], add=biasb_t[:, e:e + 1])
                    nc.sync.dma_start(out=outT[ts(e, 128), ts(tb, 512)], in_=ot[:])

    nc.compile()
    return nc


def _prep_inputs(tokens, token_vectors, W, b):
    tokens = np.asarray(tokens)
    table = np.ascontiguousarray(np.asarray(token_vectors, dtype=np.float32))
    W = np.asarray(W, dtype=np.float32)
    b = np.asarray(b, dtype=np.float32)

    wt = np.ascontiguousarray((W / 4.0).T)          # [d, e]
    wtc = wt.reshape(128, 8 * D).copy()             # d = 8p + c -> [p, (c e)]
    wtc = wt.reshape(128, 8, D).reshape(128, 8 * D) # same thing, explicit
    wtc = wtc.astype(ml_dtypes.bfloat16)
    biasb = np.ascontiguousarray(b.reshape(8, 128).T)  # [p, e_blk]
    # cyclic partition rotate: out[m] = in[(m-1) % 128]  (prot[k, m]=1 iff m=(k+1)%128)
    prot = np.zeros((128, 128), dtype=ml_dtypes.bfloat16)
    prot[np.arange(128), (np.arange(128) + 1) % 128] = 1

    in_maps = []
    for k in range(NCORES):
        rb, half = k // 2, k % 2
        a = half * 1024
        pos = np.arange(NGRP * 128)
        gt = a - HALO + pos
        valid = (pos < NPOS) & (gt >= 0)
        idx = np.full(NGRP * 128, OOB, dtype=np.int32)
        idx[valid] = tokens[rb, gt[valid]].astype(np.int32)
        idx = np.ascontiguousarray(idx.reshape(NGRP, 128).T)  # [p, g]
        in_maps.append({
            "table": table,
            "idx": idx,
            "wtc": wtc,
            "biasb": biasb,
            "prot": prot,
        })
    return in_maps


def kernel(tokens, token_vectors, W, b, trace=False):
    if "nc" not in _cached:
        _cached["nc"] = _build_nc()
    nc = _cached["nc"]
    in_maps = _prep_inputs(tokens, token_vectors, W, b)
    res = bass_utils.run_bass_kernel_spmd(
        nc, in_maps, core_ids=list(range(NCORES)), trace=trace,
    )
    _cached["last_result"] = res
    out = np.zeros((B, T, D), dtype=np.float32)
    for k in range(NCORES):
        rb, half = k // 2, k % 2
        out[rb, half * 1024:(half + 1) * 1024, :] = res.results[k]["outT"].T
    return out



# revision 20
# speedup vs baseline: 86.0835x; 86.0835x over previous
"""HDC encoder kernel v3 — fp8 pipeline (table/ngrams fp8, DoubleRow matmul).

Numerics: token vectors are exactly +-1 (fp8e4m3-exact); ngram products stay
+-1; bundle sums are integers in [-4,4] (fp8-exact).  W/4 is scaled by 2^14
and split into fp8 main + fp8 residual (W8 + R8), making the quantization
error ~2^-8 relative — bf16-level.  Matmuls run in DoubleRow perf mode
(fp8, two 128-row k-tiles per instruction, 2x PE throughput), accumulating
W8 and R8 terms into the same PSUM tile.  The 2^-14 descale and the bias
add happen on the host during output assembly.
"""

import numpy as np
import ml_dtypes

import concourse.bass as bass
import concourse.mybir as mybir
import concourse.tile as tile
from concourse import bacc
from concourse.bass import ts, ds
from concourse.masks import make_identity
import concourse.bass_utils as bass_utils

VOCAB = 32000
D = 1024
B, T = 4, 2048
NCORES = 8
HALO = 4
NGRP = 9
NCOL = NGRP * 128
NPOS = 1024 + HALO
ZROW = VOCAB
CHUNKS = [128, 256, 256, 256, 128]   # col widths; sum = 1024
CH = 256
NCHUNK = len(CHUNKS)
WSCALE = 2.0 ** 14

_cached = {}


def _build_nc():
    f32 = mybir.dt.float32
    bf16 = mybir.dt.bfloat16
    fp8 = mybir.dt.float8e4
    DR = mybir.MatmulPerfMode.DoubleRow
    nc = bacc.Bacc("TRN2", target_bir_lowering=False, debug=False,
                   enable_asserts=False, num_devices=NCORES)

    table = nc.dram_tensor("table", [VOCAB + 1, D], bf16, kind="ExternalInput").ap()
    idx = nc.dram_tensor("idx", [128, NGRP], mybir.dt.int32, kind="ExternalInput").ap()
    wtc8 = nc.dram_tensor("wtc8", [128, 2 * 8 * D], fp8, kind="ExternalInput").ap()
    prot = nc.dram_tensor("prot", [128, 128], bf16, kind="ExternalInput").ap()
    outT = nc.dram_tensor("outT", [D, 1024], bf16, kind="ExternalOutput").ap()

    with tile.TileContext(nc) as tc:
        with tc.tile_pool(name="cst", bufs=1) as cst, \
             tc.tile_pool(name="vf", bufs=1) as vfp, \
             tc.tile_pool(name="vbt", bufs=1) as vbtp, \
             tc.tile_pool(name="sng", bufs=1) as sng, \
             tc.tile_pool(name="otile", bufs=3) as otp, \
             tc.tile_pool(name="pst", bufs=2, space="PSUM") as pst, \
             tc.tile_pool(name="prm", bufs=2, space="PSUM") as prm, \
             tc.tile_pool(name="psm", bufs=4, space="PSUM") as psm:

            idx_t = cst.tile([128, NGRP], mybir.dt.int32)
            nc.sync.dma_start(out=idx_t[:], in_=idx[:])
            prot_t = cst.tile([128, 128], bf16)
            nc.sync.dma_start(out=prot_t[:], in_=prot[:])

            # flat [128, D] gather per group (3-dim outs corrupt on HW).
            # bf16 table/transpose (walrus rejects fp8 psum transposes);
            # the PSUM->SBUF copy converts to fp8.
            vf = {}
            for g in range(NGRP):
                vft = vfp.tile([128, D], bf16, tag=f"vf{g}", name=f"vf{g}")
                nc.gpsimd.indirect_dma_start(
                    out=vft[:],
                    out_offset=None,
                    in_=table,
                    in_offset=bass.IndirectOffsetOnAxis(ap=idx_t[:, g:g + 1], axis=0),
                    bounds_check=None,
                    oob_is_err=False,
                )
                vf[g] = vft

            # wtc8[p, s, e, c, f]: s=0 main W8, s=1 residual R8.  128KB chunks
            # staggered into DMA-wire gaps behind the gathers.
            wtc_r = wtc8.rearrange("p (s e c f) -> p s e c f", s=2, e=8, c=8)
            wtc_t = cst.tile([128, 2, 8, 8, 128], fp8)
            sched = [(0, e, 6.0 + 0.5 * e) for e in range(8)] + \
                    [(1, e, 10.0 + 0.5 * e) for e in range(8)]
            for s, e, t_us in sched:
                with tc.tile_wait_until(t_us * 1e-3):
                    nc.sync.dma_start(out=wtc_t[:, s, e, :, :], in_=wtc_r[:, s, e, :, :])

            ident = cst.tile([128, 128], bf16)
            make_identity(nc, ident[:])

            vbt = vbtp.tile([128, 8, NCOL], bf16, tag="vbt", name="vbt")
            for g in range(NGRP):
                vft = vf[g]
                for half in range(2):
                    ps = pst.tile([128, 512], bf16, tag="tps")
                    for c4 in range(4):
                        c = 4 * half + c4
                        nc.tensor.transpose(
                            out=ps[:, ts(c4, 128)],
                            in_=vft[:, ds(c, 128, 8)],
                            identity=ident[:],
                        )
                    copy_eng = nc.vector.tensor_copy if g < 3 else nc.scalar.copy
                    copy_eng(
                        out=vbt[:, 4 * half:4 * half + 4, ts(g, 128)],
                        in_=ps[:].rearrange("p (c t) -> p c t", c=4),
                    )

            s2 = sng.tile([128, 8, NCOL], bf16, tag="s2", name="s2")
            s3 = sng.tile([128, 8, NCOL], bf16, tag="s3", name="s3")
            s4 = sng.tile([128, 8, NCOL], bf16, tag="s4", name="s4")
            bsum = sng.tile([128, 8, NCOL], bf16, tag="bsum", name="bsum")
            bnd = sng.tile([128, 8, NCOL], fp8, tag="bnd", name="bnd")

            mul = nc.vector.tensor_mul
            add = nc.vector.tensor_add

            def rot(src_ap, w, name):
                pm = prm.tile([128, CH], f32, tag="rot", name=name)  # max width
                nc.tensor.matmul(out=pm[:, 0:w], lhsT=prot_t[:], rhs=src_ap,
                                 start=True, stop=True)
                return pm

            ot_full = cst.tile([128, 8, 1024], mybir.dt.bfloat16)
            chunk_base = [sum(CHUNKS[:i]) for i in range(NCHUNK)]

            def mm_eblocks(k, e_lo, e_hi):
                w = CHUNKS[k]
                t0 = chunk_base[k]
                b = HALO + t0
                for e in range(e_lo, e_hi):
                    pm = psm.tile([128, w], f32, tag="mps")
                    for s in range(2):
                        for m in range(4):
                            nc.tensor.matmul(
                                out=pm[:],
                                lhsT=wtc_t[:, s, e, 2 * m:2 * m + 2, :],
                                rhs=bnd[:, 2 * m:2 * m + 2, ds(b, w)],
                                start=(s == 0 and m == 0),
                                stop=(s == 1 and m == 3),
                                perf_mode=DR,
                            )
                    nc.scalar.copy(out=ot_full[:, e, ds(t0, w)], in_=pm[:])
                    if k == NCHUNK - 1:
                        nc.sync.dma_start(out=outT[ts(e, 128), :], in_=ot_full[:, e, :])

            for k in range(NCHUNK):
                w = CHUNKS[k]
                b = HALO + sum(CHUNKS[:k])
                C = ds(b, w)

                r2 = rot(vbt[:, 7, C], w, f"r2_{k}")
                mul(out=s2[:, 1:8, C], in0=vbt[:, 0:7, C],
                    in1=vbt[:, 1:8, ds(b - 1, w)])
                mul(out=s2[:, 0, C], in0=r2[:, 0:w], in1=vbt[:, 0, ds(b - 1, w)])
                if k > 0:
                    mm_eblocks(k - 1, 0, 3)

                r3 = rot(s2[:, 7, C], w, f"r3_{k}")
                mul(out=s3[:, 1:8, C], in0=s2[:, 0:7, C],
                    in1=vbt[:, 1:8, ds(b - 2, w)])
                mul(out=s3[:, 0, C], in0=r3[:, 0:w], in1=vbt[:, 0, ds(b - 2, w)])
                if k > 0:
                    mm_eblocks(k - 1, 3, 6)

                r4 = rot(s3[:, 7, C], w, f"r4_{k}")
                mul(out=s4[:, 1:8, C], in0=s3[:, 0:7, C],
                    in1=vbt[:, 1:8, ds(b - 3, w)])
                mul(out=s4[:, 0, C], in0=r4[:, 0:w], in1=vbt[:, 0, ds(b - 3, w)])
                if k > 0:
                    mm_eblocks(k - 1, 6, 8)

                add(out=bsum[:, :, C], in0=vbt[:, :, C], in1=s2[:, :, C])
                add(out=bsum[:, :, C], in0=bsum[:, :, C], in1=s3[:, :, C])
                add(out=bsum[:, :, C], in0=bsum[:, :, C], in1=s4[:, :, C])
                nc.gpsimd.tensor_copy(out=bnd[:, :, C], in_=bsum[:, :, C])

            mm_eblocks(NCHUNK - 1, 0, 8)

    nc.compile()
    return nc


def _prep_static(token_vectors, W, b):
    key = (id(token_vectors), id(W), id(b))
    if _cached.get("static_key") == key:
        return _cached["static"]
    e4 = ml_dtypes.float8_e4m3
    table = np.asarray(token_vectors, dtype=np.float32)
    tbl = np.zeros((VOCAB + 1, D), dtype=ml_dtypes.bfloat16)
    tbl[:VOCAB] = table.astype(ml_dtypes.bfloat16)
    W = np.asarray(W, dtype=np.float32)
    bv = np.asarray(b, dtype=np.float32)

    wt = (W / 4.0).T * WSCALE                     # [d, e], scaled to fp8 range
    W8 = wt.astype(e4)
    R8 = (wt - W8.astype(np.float32)).astype(e4)
    # [d=8p+c, e=128eb+f] -> [p, c, eb, f] -> [p, eb, c, f]
    def lay(a):
        return np.ascontiguousarray(
            a.reshape(128, 8, 8, 128).transpose(0, 2, 1, 3))
    wtc8 = np.stack([lay(W8), lay(R8)], axis=1)   # [p, s, eb, c, f]
    wtc8 = np.ascontiguousarray(wtc8.reshape(128, 2 * 8 * D))
    prot = np.zeros((128, 128), dtype=ml_dtypes.bfloat16)
    prot[np.arange(128), (np.arange(128) + 1) % 128] = 1
    st_inputs = {"table": tbl, "wtc8": wtc8, "prot": prot}
    st = {"inputs": st_inputs, "bias": bv}
    _cached["static_key"] = key
    _cached["static"] = st
    return st


def _prep_inputs(tokens, token_vectors, W, b):
    tokens = np.asarray(tokens)
    st = _prep_static(token_vectors, W, b)
    in_maps = []
    for k in range(NCORES):
        rb, half = k // 2, k % 2
        a = half * 1024
        pos = np.arange(NCOL)
        gt = a - HALO + pos
        valid = (pos < NPOS) & (gt >= 0)
        idxv = np.full(NCOL, ZROW, dtype=np.int32)
        idxv[valid] = tokens[rb, gt[valid]].astype(np.int32)
        idxv = np.ascontiguousarray(idxv.reshape(NGRP, 128).T)
        in_maps.append({"idx": idxv, **st["inputs"]})
    return in_maps


def kernel(tokens, token_vectors, W, b, trace=False):
    if "nc" not in _cached:
        _cached["nc"] = _build_nc()
    nc = _cached["nc"]
    in_maps = _prep_inputs(tokens, token_vectors, W, b)
    res = bass_utils.run_bass_kernel_spmd(
        nc, in_maps, core_ids=list(range(NCORES)), trace=trace,
    )
    _cached["last_result"] = res
    bias = _cached["static"]["bias"]
    out = np.zeros((B, T, D), dtype=np.float32)
    for k in range(NCORES):
        rb, half = k // 2, k % 2
        o = res.results[k]["outT"].astype(np.float32).T * (1.0 / WSCALE)
        out[rb, half * 1024:(half + 1) * 1024, :] = o + bias
    return out
